# revision 5
# baseline (speedup 1.0000x reference)
"""Causal multi-head attention (B=4, S=2048, D=2048, H=16) on 8 TRN2 NeuronCores.

Sharding: core c = 2*b + g handles batch b (of 4) and head-group g (of 2,
8 heads each).  Megatron-style: q/k/v projections are column-parallel over
the head dimension, the output projection is row-parallel; the host sums
the two partial outputs per batch and adds the bias.

All on-device data is bf16 (host pre-converts); matmuls run at full PE
rate at any width, so causal score/attend matmuls narrow to exact
128-column granularity.  q/k/v and the per-head attention outputs stay
resident in SBUF — no DRAM round-trips between phases.  Softmax skips the
max-subtraction (scores are ~N(0,1); exp cannot overflow): scores are
computed transposed [sk, sq], the denominator comes from a ones-vector
matmul, and normalization is applied after attn@v.
"""

import math

import numpy as np

B, S, D = 4, 2048, 2048
H_TOTAL, DH = 16, 128
G = 2               # tensor-parallel head groups
HG = H_TOTAL // G   # 8 heads per group
F = HG * DH         # 1024 features per group
N_CORES = 8

_CACHE = {}


def _build_nc(iters=1):
    import concourse.mybir as mybir
    from concourse import bacc
    from concourse.tile import TileContext
    from concourse.masks import make_upper_triangular

    BF16 = mybir.dt.bfloat16
    F32 = mybir.dt.float32
    AF = mybir.ActivationFunctionType
    MUL = mybir.AluOpType.mult

    DT = D // 128    # 16 contraction tiles
    ST = S // 128    # 16 seq tiles
    FT = F // 128    # 8 feature tiles (= heads per group)
    SB = S // 512    # 4 seq blocks
    FB = F // 256    # 4 feature quarter-blocks (v projection)

    nc = bacc.Bacc("TRN2", target_bir_lowering=False, debug=False)
    xT = nc.dram_tensor("xT", [D, S], BF16, kind="ExternalInput")
    wq = nc.dram_tensor("wq", [D, F], BF16, kind="ExternalInput")
    wk = nc.dram_tensor("wk", [D, F], BF16, kind="ExternalInput")
    wv = nc.dram_tensor("wv", [D, F], BF16, kind="ExternalInput")
    wo = nc.dram_tensor("wo", [F, D], BF16, kind="ExternalInput")
    out = nc.dram_tensor("partial", [S, D], F32, kind="ExternalOutput")

    with TileContext(nc) as tc:
        with tc.tile_pool(name="const", bufs=1) as cp:
            # 128x128 upper-triangular (incl. diagonal) mask for the causal
            # diagonal tiles of the transposed scores [sk, sq].
            mask = cp.tile([128, 128], BF16, name="mask")
            make_upper_triangular(nc, mask[:], val=1.0, diag=True)
            o32 = cp.tile([128, 1], F32)
            nc.gpsimd.memset(o32[:], 1.0)
            ones = cp.tile([128, 1], BF16)
            nc.vector.tensor_copy(ones[:], o32[:])

            for _ in range(iters):
                with tc.tile_pool(name="it", bufs=1) as pit:
                    qT = pit.tile([128, HG, S], BF16)   # [dh, h, sq]
                    kT = pit.tile([128, HG, S], BF16)   # [dh, h, sk]
                    va = pit.tile([128, ST, F], BF16)   # [sk%128, sk//128, f]

                    # ---- phase 1: q/k/v projections ---------------------
                    with (
                        tc.tile_pool(name="ph1", bufs=1) as p1,
                        tc.tile_pool(name="ps1", bufs=1, space="PSUM") as ps1,
                    ):
                        xt = p1.tile([128, DT, S], BF16)  # x.T resident

                        def load_wqk(h):
                            wqf = p1.tile([128, DT, 128], BF16, tag="wqf", bufs=2)
                            wkf = p1.tile([128, DT, 128], BF16, tag="wkf", bufs=2)
                            fs = slice(h * 128, (h + 1) * 128)
                            nc.sync.dma_start(
                                out=wqf[:],
                                in_=wq[:, fs].rearrange("(t p) f -> p t f", p=128),
                            )
                            nc.sync.dma_start(
                                out=wkf[:],
                                in_=wk[:, fs].rearrange("(t p) f -> p t f", p=128),
                            )
                            return wqf, wkf

                        # h=0 weights first, then x.T column-block by
                        # column-block, so the first matmul chain only waits
                        # on ~3MB of DMA.
                        wqk0 = load_wqk(0)
                        for sb in range(SB):
                            for d in range(DT):
                                nc.sync.dma_start(
                                    out=xt[:, d, sb * 512 : (sb + 1) * 512],
                                    in_=xT[d * 128 : (d + 1) * 128,
                                           sb * 512 : (sb + 1) * 512],
                                )

                        for h in range(HG):
                            wqf, wkf = wqk0 if h == 0 else load_wqk(h)
                            for sb in range(SB):
                                ss = slice(sb * 512, (sb + 1) * 512)
                                for w_t, dst in ((wqf, qT), (wkf, kT)):
                                    acc = ps1.tile([128, 512], F32, tag="ps_qk", bufs=2)
                                    for d in range(DT):
                                        nc.tensor.matmul(
                                            acc[:],
                                            w_t[:, d, :],
                                            xt[:, d, ss],
                                            start=(d == 0),
                                            stop=(d == DT - 1),
                                        )
                                    nc.vector.tensor_copy(dst[:, h, ss], acc[:])

                        def load_wv(hp):
                            wvb = p1.tile([128, DT, 256], BF16, tag="wvb", bufs=2)
                            fbs = slice(hp * 256, (hp + 1) * 256)
                            nc.sync.dma_start(
                                out=wvb[:],
                                in_=wv[:, fbs].rearrange("(t p) f -> p t f", p=128),
                            )
                            return wvb

                        wv_next = load_wv(0)
                        for hp in range(FB):
                            wvb = wv_next
                            if hp + 1 < FB:
                                wv_next = load_wv(hp + 1)
                            fbs = slice(hp * 256, (hp + 1) * 256)
                            for st in range(ST):
                                acc = ps1.tile([128, 256], F32, tag="ps_v", bufs=2)
                                for d in range(DT):
                                    nc.tensor.matmul(
                                        acc[:],
                                        xt[:, d, st * 128 : (st + 1) * 128],
                                        wvb[:, d, :],
                                        start=(d == 0),
                                        stop=(d == DT - 1),
                                    )
                                nc.vector.tensor_copy(va[:, st, fbs], acc[:])

                    # phases 2+3 reuse phase 1's x.T space for the attention
                    # output oT and the wo weights (loaded during attention,
                    # first used in phase 3).
                    with tc.tile_pool(name="p23", bufs=1) as p2:
                        oT = p2.tile([128, HG, S], BF16)   # [dh, h, sq]
                        wof = p2.tile([128, FT, D], BF16)
                        nc.sync.dma_start(
                            out=wof[:], in_=wo.rearrange("(t p) f -> p t f", p=128)
                        )

                        # ---- phase 2: causal attention per head ---------
                        with (
                            tc.tile_pool(name="ps2s", bufs=1, space="PSUM") as ps2s,
                            tc.tile_pool(name="ps2o", bufs=1, space="PSUM") as ps2o,
                        ):
                            # Software-pipelined by j-pairs: pair p's av/l
                            # matmuls are emitted after pair p+DEPTH's score
                            # matmuls, so the PE never sits behind p's exp
                            # (ACT) or the diagonal mask multiply (DVE).
                            DEPTH = 2
                            pend = []  # (pt, h, acc_o, acc_l, j0, b)
                            epilogue = None

                            def flush_pending():
                                nonlocal epilogue
                                if not pend:
                                    return
                                pt_, h_, acc_o_, acc_l_, j0_, b_ = pend.pop(0)
                                jmax_ = 4 * b_ + 3
                                for hh in range(2):
                                    j = j0_ + hh
                                    a = j - 4 * b_
                                    c0 = max(a, 0) * 128
                                    pslice = pt_[:, hh * 512 + c0 : (hh + 1) * 512]
                                    nc.tensor.matmul(
                                        acc_o_[:, c0:512],
                                        va[:, j, h_ * 128 : (h_ + 1) * 128],
                                        pslice,
                                        start=(j == 0),
                                        stop=(j == jmax_),
                                    )
                                    nc.tensor.matmul(
                                        acc_l_[:, c0:512],
                                        ones[:],
                                        pslice,
                                        start=(j == 0),
                                        stop=(j == jmax_),
                                    )
                                if j0_ + 1 == jmax_:
                                    epilogue = (acc_o_, acc_l_)

                            def flush_epilogue(h_, b_):
                                nonlocal epilogue
                                acc_o_, acc_l_ = epilogue
                                epilogue = None
                                bs_ = slice(b_ * 512, (b_ + 1) * 512)
                                linv = p2.tile([1, 512], F32, tag="linv", bufs=2)
                                nc.vector.reciprocal(linv[:], acc_l_[:])
                                linb = p2.tile([128, 512], F32, tag="linb", bufs=2)
                                nc.gpsimd.partition_broadcast(linb[:], linv[:])
                                nc.vector.tensor_tensor(
                                    out=oT[:, h_, bs_], in0=acc_o_[:],
                                    in1=linb[:], op=MUL,
                                )

                            blocks = []  # (h, b) epilogue coords in flight
                            for h in range(HG):
                                for b in range(SB):
                                    acc_o = ps2o.tile(
                                        [128, 512], F32, tag="ps_o", bufs=2
                                    )
                                    acc_l = ps2o.tile(
                                        [1, 512], F32, tag="ps_l", bufs=2
                                    )
                                    for jp in range(2 * b + 2):
                                        j0 = 2 * jp
                                        sc = ps2s.tile(
                                            [128, 1024], F32, tag="ps_s", bufs=2
                                        )
                                        for hh in range(2):
                                            j = j0 + hh
                                            a = j - 4 * b
                                            c0 = max(a, 0) * 128
                                            nc.tensor.matmul(
                                                sc[:, hh * 512 + c0 : (hh + 1) * 512],
                                                kT[:, h, j * 128 : (j + 1) * 128],
                                                qT[:, h, b * 512 + c0 : (b + 1) * 512],
                                                start=True,
                                                stop=True,
                                            )
                                        cp0 = max(j0 - 4 * b, 0) * 128
                                        pt = p2.tile(
                                            [128, 1024], BF16, tag="pt", bufs=4
                                        )
                                        nc.scalar.activation(
                                            pt[:, cp0:1024], sc[:, cp0:1024], AF.Exp
                                        )
                                        if j0 >= 4 * b:  # diagonal pair
                                            for hh in range(2):
                                                a = j0 + hh - 4 * b
                                                ms = hh * 512 + a * 128
                                                nc.vector.tensor_tensor(
                                                    out=pt[:, ms : ms + 128],
                                                    in0=pt[:, ms : ms + 128],
                                                    in1=mask[:],
                                                    op=MUL,
                                                )
                                        pend.append((pt, h, acc_o, acc_l, j0, b))
                                        if len(pend) > DEPTH:
                                            flush_pending()
                                            if epilogue is not None:
                                                flush_epilogue(*blocks.pop(0))
                                    blocks.append((h, b))
                            while pend:
                                flush_pending()
                                if epilogue is not None:
                                    flush_epilogue(*blocks.pop(0))

                        # ---- phase 3: output projection -----------------
                        with tc.tile_pool(name="ps3", bufs=1, space="PSUM") as ps3:
                            for st in range(ST):
                                sts = slice(st * 128, (st + 1) * 128)
                                for ob in range(SB):
                                    obs = slice(ob * 512, (ob + 1) * 512)
                                    acc = ps3.tile(
                                        [128, 512], F32, tag="ps_p", bufs=2
                                    )
                                    for f in range(FT):
                                        nc.tensor.matmul(
                                            acc[:],
                                            oT[:, f, sts],
                                            wof[:, f, obs],
                                            start=(f == 0),
                                            stop=(f == FT - 1),
                                        )
                                    po = p2.tile([128, 512], F32, tag="po", bufs=3)
                                    nc.vector.tensor_copy(po[:], acc[:])
                                    nc.sync.dma_start(out=out[sts, obs], in_=po[:])

    nc.compile()
    return nc


def _get_nc(iters=1):
    key = ("nc", iters)
    if key not in _CACHE:
        _CACHE[key] = _build_nc(iters)
    return _CACHE[key]


def make_in_maps(x, Wq, Wk, Wv, Wo):
    import ml_dtypes

    bf16 = np.dtype(ml_dtypes.bfloat16)
    scale = np.float32(1.0 / math.sqrt(DH))
    xTs = [np.ascontiguousarray(x[b].T).astype(bf16) for b in range(B)]
    in_maps = []
    for c in range(N_CORES):
        b, g = divmod(c, G)
        gs = slice(g * F, (g + 1) * F)
        in_maps.append(
            {
                "xT": xTs[b],
                "wq": (np.ascontiguousarray(Wq[gs, :].T) * scale).astype(bf16),
                "wk": np.ascontiguousarray(Wk[gs, :].T).astype(bf16),
                "wv": np.ascontiguousarray(Wv[gs, :].T).astype(bf16),
                "wo": np.ascontiguousarray(Wo[:, gs].T).astype(bf16),
            }
        )
    return in_maps


def kernel(x, Wq, Wk, Wv, Wo, bo):
    from concourse.bass_utils import run_bass_kernel_spmd

    x = np.asarray(x, dtype=np.float32)
    Wq = np.asarray(Wq, dtype=np.float32)
    Wk = np.asarray(Wk, dtype=np.float32)
    Wv = np.asarray(Wv, dtype=np.float32)
    Wo = np.asarray(Wo, dtype=np.float32)
    bo = np.asarray(bo, dtype=np.float32)

    nc = _get_nc()
    in_maps = make_in_maps(x, Wq, Wk, Wv, Wo)
    res = run_bass_kernel_spmd(nc, in_maps, list(range(N_CORES)))
    out = np.empty((B, S, D), dtype=np.float32)
    for b in range(B):
        out[b] = res.results[2 * b]["partial"] + res.results[2 * b + 1]["partial"] + bo
    return out


# revision 10
# speedup vs baseline: 1.0846x; 1.0846x over previous
"""Causal multi-head attention (B=4, S=2048, D=2048, H=16) on 8 TRN2 NeuronCores.

Sharding: core c = 2*b + g handles batch b (of 4) and head-group g (of 2,
8 heads each).  Megatron-style: q/k/v projections are column-parallel over
the head dimension, the output projection is row-parallel; the host sums
the two partial outputs per batch and adds the bias.

All on-device data is bf16 (host pre-converts); matmuls run at full PE
rate at any width, so causal score/attend matmuls narrow to exact
128-column granularity.  q/k/v and the per-head attention outputs stay
resident in SBUF — no DRAM round-trips between phases.  Softmax skips the
max-subtraction (scores are ~N(0,1); exp cannot overflow): scores are
computed transposed [sk, sq], the denominator comes from a ones-vector
matmul, and normalization is applied after attn@v.
"""

import math

import numpy as np

B, S, D = 4, 2048, 2048
H_TOTAL, DH = 16, 128
G = 2               # tensor-parallel head groups
HG = H_TOTAL // G   # 8 heads per group
F = HG * DH         # 1024 features per group
N_CORES = 8

_CACHE = {}


def _build_nc(iters=1, attn_reps=1):
    import concourse.mybir as mybir
    from concourse import bacc
    from concourse.tile import TileContext
    from concourse.masks import make_upper_triangular

    BF16 = mybir.dt.bfloat16
    F32 = mybir.dt.float32
    AF = mybir.ActivationFunctionType
    MUL = mybir.AluOpType.mult

    DT = D // 128    # 16 contraction tiles
    ST = S // 128    # 16 seq tiles
    FT = F // 128    # 8 feature tiles (= heads per group)
    SB = S // 512    # 4 seq blocks
    FB = F // 256    # 4 feature quarter-blocks (v projection)

    nc = bacc.Bacc("TRN2", target_bir_lowering=False, debug=False)
    # all inputs are pre-tiled on the host into their exact SBUF layouts so
    # every DMA is one fully-contiguous run per partition
    xT = nc.dram_tensor("xT", [128, SB, DT, 512], BF16, kind="ExternalInput")
    wq = nc.dram_tensor("wq", [128, HG, DT, 128], BF16, kind="ExternalInput")
    wk = nc.dram_tensor("wk", [128, HG, DT, 128], BF16, kind="ExternalInput")
    wv = nc.dram_tensor("wv", [128, FB, DT, 256], BF16, kind="ExternalInput")
    wo = nc.dram_tensor("wo", [128, FT, D], BF16, kind="ExternalInput")
    out = nc.dram_tensor("partial", [S, D], F32, kind="ExternalOutput")

    with TileContext(nc) as tc:
        with tc.tile_pool(name="const", bufs=1) as cp:
            # 128x128 upper-triangular (incl. diagonal) mask for the causal
            # diagonal tiles of the transposed scores [sk, sq].
            mask = cp.tile([128, 128], BF16, name="mask")
            make_upper_triangular(nc, mask[:], val=1.0, diag=True)
            # [128,128] all-ones stationary: the denominator matmul then
            # writes the column sum to EVERY output partition (same cost --
            # PE time depends only on N), so no partition broadcast is
            # needed downstream.
            ones = cp.tile([128, 128], BF16)
            nc.gpsimd.memset(ones[:], 1.0)

            for _ in range(iters):
                with tc.tile_pool(name="it", bufs=1) as pit:
                    qT = pit.tile([128, HG, S], BF16)   # [dh, h, sq]
                    kT = pit.tile([128, HG, S], BF16)   # [dh, h, sk]
                    va = pit.tile([128, ST, F], BF16)   # [sk%128, sk//128, f]

                    # ---- phase 1: q/k/v projections ---------------------
                    with (
                        tc.tile_pool(name="ph1", bufs=1) as p1,
                        tc.tile_pool(name="ps1", bufs=1, space="PSUM") as ps1,
                    ):
                        xt = p1.tile([128, DT, S], BF16)  # x.T resident

                        def load_wqk(h):
                            wqf = p1.tile([128, DT, 128], BF16, tag="wqf", bufs=2)
                            wkf = p1.tile([128, DT, 128], BF16, tag="wkf", bufs=2)
                            nc.sync.dma_start(out=wqf[:], in_=wq[:, h, :, :])
                            nc.sync.dma_start(out=wkf[:], in_=wk[:, h, :, :])
                            return wqf, wkf

                        # h=0 weights first, then x.T column-block by
                        # column-block, so the first matmul chain only waits
                        # on ~3MB of DMA.
                        wqk0 = load_wqk(0)
                        for sb in range(SB):
                            nc.sync.dma_start(
                                out=xt[:, :, sb * 512 : (sb + 1) * 512],
                                in_=xT[:, sb, :, :],
                            )

                        for h in range(HG):
                            wqf, wkf = wqk0 if h == 0 else load_wqk(h)
                            for sb in range(SB):
                                ss = slice(sb * 512, (sb + 1) * 512)
                                for w_t, dst in ((wqf, qT), (wkf, kT)):
                                    acc = ps1.tile([128, 512], F32, tag="ps_qk", bufs=2)
                                    for d in range(DT):
                                        nc.tensor.matmul(
                                            acc[:],
                                            w_t[:, d, :],
                                            xt[:, d, ss],
                                            start=(d == 0),
                                            stop=(d == DT - 1),
                                        )
                                    nc.vector.tensor_copy(dst[:, h, ss], acc[:])

                        def load_wv(hp):
                            wvb = p1.tile([128, DT, 256], BF16, tag="wvb", bufs=2)
                            nc.sync.dma_start(out=wvb[:], in_=wv[:, hp, :, :])
                            return wvb

                        wv_next = load_wv(0)
                        for hp in range(FB):
                            wvb = wv_next
                            if hp + 1 < FB:
                                wv_next = load_wv(hp + 1)
                            fbs = slice(hp * 256, (hp + 1) * 256)
                            for st in range(ST):
                                acc = ps1.tile([128, 256], F32, tag="ps_v", bufs=2)
                                for d in range(DT):
                                    nc.tensor.matmul(
                                        acc[:],
                                        xt[:, d, st * 128 : (st + 1) * 128],
                                        wvb[:, d, :],
                                        start=(d == 0),
                                        stop=(d == DT - 1),
                                    )
                                nc.vector.tensor_copy(va[:, st, fbs], acc[:])

                    # phases 2+3 reuse phase 1's x.T space for the attention
                    # output oT and the wo weights (loaded during attention,
                    # first used in phase 3).
                    with tc.tile_pool(name="p23", bufs=1) as p2:
                        oT = p2.tile([128, HG, S], BF16)   # [dh, h, sq]
                        wof = p2.tile([128, FT, D], BF16)
                        nc.sync.dma_start(out=wof[:], in_=wo[:])

                        # ---- phase 2: causal attention per head ---------
                        with (
                            tc.tile_pool(name="ps2s", bufs=1, space="PSUM") as ps2s,
                            tc.tile_pool(name="ps2o", bufs=1, space="PSUM") as ps2o,
                        ):
                            # Software-pipelined by j-pairs: pair p's av/l
                            # matmuls are emitted after pair p+DEPTH's score
                            # matmuls, so the PE never sits behind p's exp
                            # (ACT) or the diagonal mask multiply (DVE).
                            DEPTH = 2
                            pend = []  # (pt, h, acc_o, acc_l, j0, b)
                            epilogue = None

                            def flush_pending():
                                nonlocal epilogue
                                if not pend:
                                    return
                                pt_, h_, acc_o_, acc_l_, j0_, b_ = pend.pop(0)
                                jmax_ = 4 * b_ + 3
                                for hh in range(2):
                                    j = j0_ + hh
                                    a = j - 4 * b_
                                    c0 = max(a, 0) * 128
                                    pslice = pt_[:, hh * 512 + c0 : (hh + 1) * 512]
                                    nc.tensor.matmul(
                                        acc_o_[:, c0:512],
                                        va[:, j, h_ * 128 : (h_ + 1) * 128],
                                        pslice,
                                        start=(j == 0),
                                        stop=(j == jmax_),
                                    )
                                    nc.tensor.matmul(
                                        acc_l_[:, c0:512],
                                        ones[:],
                                        pslice,
                                        start=(j == 0),
                                        stop=(j == jmax_),
                                    )
                                if j0_ + 1 == jmax_:
                                    epilogue = (acc_o_, acc_l_)

                            def flush_epilogue(h_, b_):
                                nonlocal epilogue
                                acc_o_, acc_l_ = epilogue
                                epilogue = None
                                bs_ = slice(b_ * 512, (b_ + 1) * 512)
                                linv = p2.tile([128, 512], F32, tag="linv", bufs=2)
                                nc.vector.reciprocal(linv[:], acc_l_[:])
                                nc.vector.tensor_tensor(
                                    out=oT[:, h_, bs_], in0=acc_o_[:],
                                    in1=linv[:], op=MUL,
                                )

                            blocks = []  # (h, b) epilogue coords in flight
                            if attn_reps == 0:
                                nc.gpsimd.memset(oT[:], 0.03)
                            heads = [
                                h for _r in range(attn_reps) for h in range(HG)
                            ]
                            for h in heads:
                                for b in range(SB):
                                    acc_o = ps2o.tile(
                                        [128, 512], F32, tag="ps_o", bufs=2
                                    )
                                    acc_l = ps2o.tile(
                                        [128, 512], F32, tag="ps_l", bufs=2
                                    )
                                    for jp in range(2 * b + 2):
                                        j0 = 2 * jp
                                        sc = ps2s.tile(
                                            [128, 1024], F32, tag="ps_s", bufs=2
                                        )
                                        cp0 = max(j0 - 4 * b, 0) * 128
                                        for hh in range(2):
                                            j = j0 + hh
                                            nc.tensor.matmul(
                                                sc[:, hh * 512 + cp0 : (hh + 1) * 512],
                                                kT[:, h, j * 128 : (j + 1) * 128],
                                                qT[:, h, b * 512 + cp0 : (b + 1) * 512],
                                                start=True,
                                                stop=True,
                                            )
                                        pt = p2.tile(
                                            [128, 1024], BF16, tag="pt", bufs=4
                                        )
                                        if cp0:
                                            # diagonal pair whose window
                                            # crosses the half boundary: the
                                            # strip [512:512+cp0) is unwritten
                                            nc.scalar.activation(
                                                pt[:, cp0:512], sc[:, cp0:512],
                                                AF.Exp,
                                            )
                                            nc.scalar.activation(
                                                pt[:, 512 + cp0 : 1024],
                                                sc[:, 512 + cp0 : 1024],
                                                AF.Exp,
                                            )
                                        else:
                                            nc.scalar.activation(
                                                pt[:, 0:1024], sc[:, 0:1024],
                                                AF.Exp,
                                            )
                                        if j0 >= 4 * b:  # diagonal pair
                                            for hh in range(2):
                                                a = j0 + hh - 4 * b
                                                ms = hh * 512 + a * 128
                                                nc.vector.tensor_tensor(
                                                    out=pt[:, ms : ms + 128],
                                                    in0=pt[:, ms : ms + 128],
                                                    in1=mask[:],
                                                    op=MUL,
                                                )
                                        pend.append((pt, h, acc_o, acc_l, j0, b))
                                        if len(pend) > DEPTH:
                                            flush_pending()
                                            if epilogue is not None:
                                                flush_epilogue(*blocks.pop(0))
                                    blocks.append((h, b))
                            while pend:
                                flush_pending()
                                if epilogue is not None:
                                    flush_epilogue(*blocks.pop(0))

                        # ---- phase 3: output projection -----------------
                        with tc.tile_pool(name="ps3", bufs=1, space="PSUM") as ps3:
                            for st in range(ST):
                                sts = slice(st * 128, (st + 1) * 128)
                                for ob in range(SB):
                                    obs = slice(ob * 512, (ob + 1) * 512)
                                    acc = ps3.tile(
                                        [128, 512], F32, tag="ps_p", bufs=2
                                    )
                                    for f in range(FT):
                                        nc.tensor.matmul(
                                            acc[:],
                                            oT[:, f, sts],
                                            wof[:, f, obs],
                                            start=(f == 0),
                                            stop=(f == FT - 1),
                                        )
                                    po = p2.tile([128, 512], F32, tag="po", bufs=3)
                                    nc.vector.tensor_copy(po[:], acc[:])
                                    nc.sync.dma_start(out=out[sts, obs], in_=po[:])

    nc.compile()
    return nc


def _get_nc(iters=1):
    key = ("nc", iters)
    if key not in _CACHE:
        _CACHE[key] = _build_nc(iters)
    return _CACHE[key]


def _tile_w(wT, chunk):
    """[D, F] -> [128, F//chunk, D//128, chunk] contiguous (SBUF layout)."""
    d, f = wT.shape
    return np.ascontiguousarray(
        wT.reshape(d // 128, 128, f // chunk, chunk).transpose(1, 2, 0, 3)
    )


def make_in_maps(x, Wq, Wk, Wv, Wo):
    import ml_dtypes

    bf16 = np.dtype(ml_dtypes.bfloat16)
    scale = np.float32(1.0 / math.sqrt(DH))
    xTs = [
        np.ascontiguousarray(
            x[b].T.reshape(D // 128, 128, S // 512, 512).transpose(1, 2, 0, 3)
        ).astype(bf16)
        for b in range(B)
    ]
    in_maps = []
    for c in range(N_CORES):
        b, g = divmod(c, G)
        gs = slice(g * F, (g + 1) * F)
        woT = Wo[:, gs].T  # [F, D]
        in_maps.append(
            {
                "xT": xTs[b],
                "wq": _tile_w(Wq[gs, :].T * scale, 128).astype(bf16),
                "wk": _tile_w(Wk[gs, :].T, 128).astype(bf16),
                "wv": _tile_w(Wv[gs, :].T, 256).astype(bf16),
                "wo": np.ascontiguousarray(
                    woT.reshape(F // 128, 128, D).transpose(1, 0, 2)
                ).astype(bf16),
            }
        )
    return in_maps


def kernel(x, Wq, Wk, Wv, Wo, bo):
    from concourse.bass_utils import run_bass_kernel_spmd

    x = np.asarray(x, dtype=np.float32)
    Wq = np.asarray(Wq, dtype=np.float32)
    Wk = np.asarray(Wk, dtype=np.float32)
    Wv = np.asarray(Wv, dtype=np.float32)
    Wo = np.asarray(Wo, dtype=np.float32)
    bo = np.asarray(bo, dtype=np.float32)

    nc = _get_nc()
    in_maps = make_in_maps(x, Wq, Wk, Wv, Wo)
    res = run_bass_kernel_spmd(nc, in_maps, list(range(N_CORES)))
    out = np.empty((B, S, D), dtype=np.float32)
    for b in range(B):
        out[b] = res.results[2 * b]["partial"] + res.results[2 * b + 1]["partial"] + bo
    return out
